# revision 16
# baseline (speedup 1.0000x reference)
"""DRMM histogram-binning kernel for 8 Trainium2 NeuronCores.

Sharding: pure data parallel over the batch dim (64 batches -> 8 cores x 8).
Each core: cosine interaction [8,32,4096] via fp16 PE matmuls, histogram via
threshold counting (29 passes split across DVE/ACT), log1p + tiny FFN +
gated masked softmax, all on-device. Host only shards inputs / concats.

v2 layout: document loaded as one fully-contiguous [128, 32*300] f16 tile
(doc row d = 32*p + c; histogram is doc-permutation invariant), norms via
fused square+accum chunks on DVE, rsqrt via single ACT Rsqrt, normalize on
GPSIMD (frees DVE for hist counting), padded-dn pad columns memset once.
"""

import numpy as np

import concourse.bass as bass
import concourse.bacc as bacc
import concourse.mybir as mybir
import concourse.tile as tile
from concourse.bass_utils import run_bass_kernel_spmd

F32 = mybir.dt.float32
F16 = mybir.dt.float16
F8 = mybir.dt.float8e4
AF = mybir.ActivationFunctionType
ALU = mybir.AluOpType

NB = 8      # batches per core
Q = 32      # queries per batch
D = 4096    # docs per batch
E = 300     # embedding dim
EP = 384    # E padded to 3*128
NC_CH = 32  # doc chunks of 128 per batch
NBINS = 30
EPS = 1e-5

# ---- engine work splits (tunable) ----
DVE_C = list(range(0, 16))    # norm-square chunks on DVE
ACT_C = list(range(16, 32))   # norm-square chunks on ACT
DVE_J = list(range(1, 15))    # hist thresholds on DVE (is_ge counts)
ACT_J = list(range(15, 30))   # hist thresholds on ACT (sign sums), contiguous!

_THR = [j / 15.0 - 1.0 for j in range(31)]


def build_program(nc: bass.Bass):
    # ---------------- DRAM I/O ----------------
    query = nc.dram_tensor("query", [NB, Q, E], F32, kind="ExternalInput").ap()
    document = nc.dram_tensor("document", [NB, D, E], F32, kind="ExternalInput").ap()
    query_mask = nc.dram_tensor("query_mask", [NB, Q], F32, kind="ExternalInput").ap()
    q_idf = nc.dram_tensor("q_idf", [NB, Q], F32, kind="ExternalInput").ap()
    w1 = nc.dram_tensor("w1", [5, NBINS], F32, kind="ExternalInput").ap()
    b1 = nc.dram_tensor("b1", [5], F32, kind="ExternalInput").ap()
    w2 = nc.dram_tensor("w2", [1, 5], F32, kind="ExternalInput").ap()
    b2 = nc.dram_tensor("b2", [1], F32, kind="ExternalInput").ap()
    w3 = nc.dram_tensor("w3", [1, 1], F32, kind="ExternalInput").ap()
    b3 = nc.dram_tensor("b3", [1], F32, kind="ExternalInput").ap()
    gw = nc.dram_tensor("gw", [1, 1], F32, kind="ExternalInput").ap()
    gb = nc.dram_tensor("gb", [1], F32, kind="ExternalInput").ap()
    out = nc.dram_tensor("out", [NB], F32, kind="ExternalOutput").ap()

    with tile.TileContext(nc) as tc:
        with (
            tc.tile_pool(name="consts", bufs=1) as cpool,
            tc.tile_pool(name="big", bufs=1) as bigp,
            tc.tile_pool(name="misc", bufs=2) as misc,
            tc.tile_pool(name="hist", bufs=1) as hp,
        ):
            # ---------------- constants / small inputs ----------------
            w1T = cpool.tile([NBINS, 5], F16, name="w1T")
            nc.gpsimd.dma_start(w1T, w1.rearrange("a b -> b a"))
            b1s = cpool.tile([5, 1], F32, name="b1s")
            nc.sync.dma_start(b1s, b1.rearrange("(a b) -> a b", b=1))
            w2T = cpool.tile([5, 1], F32, name="w2T")
            nc.sync.dma_start(w2T, w2.rearrange("a b -> b a"))
            b2s = cpool.tile([128, 1], F32, name="b2s")
            nc.sync.dma_start(b2s, b2.rearrange("(a b) -> a b", b=1).broadcast_to([128, 1]))
            w3s = cpool.tile([128, 1], F32, name="w3s")
            nc.sync.dma_start(w3s, w3.broadcast_to([128, 1]))
            b3s = cpool.tile([128, 1], F32, name="b3s")
            nc.sync.dma_start(b3s, b3.rearrange("(a b) -> a b", b=1).broadcast_to([128, 1]))
            gws = cpool.tile([128, 1], F32, name="gws")
            nc.sync.dma_start(gws, gw.broadcast_to([128, 1]))
            gbs = cpool.tile([128, 1], F32, name="gbs")
            nc.sync.dma_start(gbs, gb.rearrange("(a b) -> a b", b=1).broadcast_to([128, 1]))

            # negated thresholds for ACT sign bias: col j = -t_j
            nthr = cpool.tile([128, 32], F32, name="nthr")
            nc.gpsimd.memset(nthr, 0.0)
            for j in ACT_J:
                nc.gpsimd.memset(nthr[:, j : j + 1], -_THR[j])

            # block-ones for the per-batch partition reduction: [128, 4]
            bones = cpool.tile([128, 4], F32, name="bones")
            nc.gpsimd.memset(bones, 0.0)
            for b in range(4):
                nc.gpsimd.memset(bones[32 * b : 32 * b + 32, b : b + 1], 1.0)

            # per-group masks / idf: [128, 1]
            qm_g, qidf_g = [], []
            qm_flat = query_mask.rearrange("b q -> (b q)")
            qidf_flat = q_idf.rearrange("b q -> (b q)")
            for g in range(2):
                qm = cpool.tile([128, 1], F32, name=f"qm{g}")
                nc.sync.dma_start(qm, qm_flat[128 * g : 128 * (g + 1)].rearrange("(p o) -> p o", o=1))
                qm_g.append(qm)
                qi = cpool.tile([128, 1], F32, name=f"qi{g}")
                nc.sync.dma_start(qi, qidf_flat[128 * g : 128 * (g + 1)].rearrange("(p o) -> p o", o=1))
                qidf_g.append(qi)

            # ---------------- query prep (both groups) ----------------
            qnT_g = []
            scr_q = hp.tile([128, E], F16, name="scr_q")
            for g in range(2):
                q_nat = cpool.tile([128, EP], F16, name=f"qnat{g}")
                nc.gpsimd.memset(q_nat, 0.0)
                qv = query.rearrange("b q e -> (b q) e")[128 * g : 128 * (g + 1), :]
                nc.gpsimd.dma_start(q_nat[:, 0:E], qv)  # f32 -> f16 cast DMA
                qn2 = cpool.tile([128, 1], F32, name=f"qn2{g}")
                nc.vector.scalar_tensor_tensor(
                    scr_q, q_nat[:, 0:E], 1.0, q_nat[:, 0:E], ALU.mult, ALU.mult,
                    accum_out=qn2,
                )
                qsq = cpool.tile([128, 1], F32, name=f"qsq{g}")
                nc.scalar.activation(qsq, qn2, AF.Sqrt, bias=0.0, scale=1.0)
                invq = cpool.tile([128, 1], F32, name=f"invq{g}")
                nc.vector.reciprocal(invq, qsq)
                qn_f16 = cpool.tile([128, EP], F16, name=f"qnf{g}")
                nc.gpsimd.memset(qn_f16, 0.0)
                nc.vector.tensor_scalar(qn_f16[:, 0:E], q_nat[:, 0:E], invq, None, ALU.mult)
                qnT = cpool.tile([128, 3 * 128], F16, name=f"qnT{g}")
                nc.sync.dma_start_transpose(qnT.rearrange("a (em q) -> a em q", em=3), qn_f16)
                qnT_g.append(qnT)

            # ---------------- big per-batch tiles (manual double buffer) ----
            doc_t = [bigp.tile([128, NC_CH * E], F16, name=f"doc{i}") for i in range(3)]
            dn_t = [bigp.tile([128, NC_CH * EP], F16, name=f"dn{i}") for i in range(2)]
            dnT_t = [bigp.tile([128, NC_CH * EP], F16, name=f"dnT{i}") for i in range(2)]
            # zero the pad columns of dn once; normalize only writes [*, 0:E]
            for i in range(2):
                nc.vector.memset(dn_t[i].rearrange("p (c e) -> p c e", e=EP)[:, :, E:EP], 0.0)

            # ---------------- scratch / hist state ----------------
            scr_dve = hp.tile([128, D], F16, name="scr_dve")
            scr_act = hp.tile([128, D], F8, name="scr_act")
            xg_t = [hp.tile([128, D], F16, name=f"xg{g}") for g in range(2)]
            C_t = [hp.tile([128, 32], F32, name=f"C{g}") for g in range(2)]
            S_t = [hp.tile([128, 32], F32, name=f"S{g}") for g in range(2)]

            # ---------------- main per-batch pipeline ----------------
            for g in range(2):
                with tc.tile_pool(name=f"mmps{g}", bufs=2, space="PSUM") as mmps:
                    ps_half = [
                        mmps.tile([128, 2048], F32, name=f"ps{g}h{h}", tag="mmps")
                        for h in range(2)
                    ]
                    for bb in range(4):
                        b = 4 * g + bb
                        doc = doc_t[b % 3]
                        docv = doc.rearrange("p (c e) -> p c e", e=E)
                        # fully contiguous load: doc row d = 32*p + c
                        nc.gpsimd.dma_start(
                            doc, document[b].rearrange("(p c) e -> p (c e)", p=128)
                        )  # f32 -> f16 cast DMA
                        # squared norms per doc-chunk (fused square+accum)
                        n2 = misc.tile([128, 32], F32, name="n2")
                        for c in DVE_C:
                            nc.vector.scalar_tensor_tensor(
                                scr_dve[:, 0:E], docv[:, c, :], 1.0,
                                docv[:, c, :], ALU.mult, ALU.mult,
                                accum_out=n2[:, c : c + 1],
                            )
                        for c in ACT_C:
                            nc.scalar.activation(
                                scr_act[:, 0:E], docv[:, c, :], AF.Square,
                                bias=0.0, scale=1.0, accum_out=n2[:, c : c + 1],
                            )
                        dsq = misc.tile([128, 32], F32, name="dsq")
                        nc.scalar.activation(dsq, n2, AF.Sqrt, bias=0.0, scale=1.0)
                        invd = misc.tile([128, 32], F16, name="invd")
                        with nc.allow_low_precision(reason="invd feeds f16 normalize"):
                            nc.vector.reciprocal(invd, dsq)
                        # normalize -> dn (f16) on GPSIMD, pad cols pre-zeroed
                        dn = dn_t[b % 2]
                        dnv = dn.rearrange("p (c e) -> p c e", e=EP)
                        nc.gpsimd.tensor_tensor(
                            dnv[:, :, 0:E], docv,
                            invd.unsqueeze(2).broadcast_to([128, 32, E]),
                            ALU.mult,
                        )
                        # big transpose: dnT[a, (c,em,p)] with partition a = e%128
                        dnT = dnT_t[b % 2]
                        nc.sync.dma_start_transpose(
                            dnT.rearrange("a (m p) -> a m p", p=128), dn
                        )
                        dnTv = dnT.rearrange("a (c em p) -> a c em p", c=NC_CH, em=3)
                        # interaction matmuls: out rows 32*bb..+32 of ps_half
                        qnT = qnT_g[g]
                        for h in range(2):
                            for nb in range(4):
                                d0 = h * 2048 + nb * 512
                                c0 = d0 // 128
                                for em in range(3):
                                    nc.tensor.matmul(
                                        ps_half[h][32 * bb : 32 * bb + 32, nb * 512 : (nb + 1) * 512],
                                        qnT[:, em * 128 + 32 * bb : em * 128 + 32 * bb + 32],
                                        dnTv[:, c0 : c0 + 4, em, :],
                                        start=(em == 0), stop=(em == 2),
                                        tile_position=(0, 32 * bb),
                                    )
                    # PSUM -> SBUF (f32 -> f16) interaction copies
                    for h in range(2):
                        nc.scalar.copy(xg_t[g][:, h * 2048 : (h + 1) * 2048], ps_half[h])

            # ---------------- histogram: threshold counting ----------------
            # issued after BOTH groups' batch pipelines so the counting
            # passes fill engine idle time instead of blocking the next
            # group's norm/normalize chain on the same queues.
            for g in range(2):
                xg = xg_t[g]
                C = C_t[g]
                S = S_t[g]
                for j in DVE_J:
                    nc.vector.tensor_scalar(
                        scr_dve, xg, _THR[j], None, ALU.is_ge, ALU.add,
                        accum_out=C[:, j : j + 1],
                    )
                for j in ACT_J:
                    nc.scalar.activation(
                        scr_act, xg, AF.Sign, bias=nthr[:, j : j + 1], scale=1.0,
                        accum_out=S[:, j : j + 1],
                    )
                # convert ACT sign-sums to counts: C = (S + D) / 2   (contiguous cols)
                ja, jb = ACT_J[0], ACT_J[-1] + 1
                nc.vector.tensor_scalar(
                    C[:, ja:jb], S[:, ja:jb], float(D), 0.5, ALU.add, ALU.mult
                )

            # ---------------- hist -> log1p -> FFN -> gated softmax ----------------
            with tc.tile_pool(name="ffnps", bufs=1, space="PSUM") as ffnps:
              psZ1 = ffnps.tile([5, 128], F32, name="psZ1")
              psZ2 = ffnps.tile([128, 1], F32, name="psZ2")
              psN = ffnps.tile([4, 1], F32, name="psN")
              psDen = ffnps.tile([4, 1], F32, name="psDen")
              for g in range(2):
                C = C_t[g]
                H = hp.tile([128, 32], F32, name=f"H{g}")
                nc.vector.tensor_tensor(H[:, 1:29], C[:, 1:29], C[:, 2:30], ALU.subtract)
                nc.vector.tensor_scalar(H[:, 0:1], C[:, 1:2], -1.0, float(D), ALU.mult, ALU.add)
                nc.vector.tensor_copy(H[:, 29:30], C[:, 29:30])
                # h = log1p(hist), f16, padded to 128 cols for the transpose
                hf = hp.tile([128, 128], F16, name=f"hf{g}")
                nc.gpsimd.memset(hf, 0.0)
                nc.scalar.activation(hf[:, 0:NBINS], H[:, 0:NBINS], AF.Ln, bias=1.0, scale=1.0)
                hT = hp.tile([128, 128], F16, name=f"hT{g}")
                nc.sync.dma_start_transpose(hT, hf)
                # z1 = tanh(w1 @ hT + b1): [5, 128]
                nc.tensor.matmul(psZ1, w1T, hT[0:NBINS, :], start=True, stop=True)
                z1 = hp.tile([5, 128], F32, name=f"z1{g}")
                nc.scalar.activation(z1, psZ1, AF.Tanh, bias=b1s, scale=1.0)
                # z2 = tanh(z1.T @ w2T + b2): [128, 1]
                nc.tensor.matmul(psZ2, z1, w2T, start=True, stop=True)
                z2b = hp.tile([128, 1], F32, name=f"z2b{g}")
                nc.scalar.activation(z2b, psZ2, AF.Tanh, bias=b2s, scale=1.0)
                zf = hp.tile([128, 1], F32, name=f"zf{g}")
                nc.scalar.activation(zf, z2b, AF.Tanh, bias=b3s, scale=w3s)
                # gate: exp(tanh(idf*gw + gb)) * mask
                g1 = hp.tile([128, 1], F32, name=f"g1{g}")
                nc.scalar.activation(g1, qidf_g[g], AF.Tanh, bias=gbs, scale=gws)
                ge = hp.tile([128, 1], F32, name=f"ge{g}")
                nc.scalar.activation(ge, g1, AF.Exp, bias=0.0, scale=1.0)
                gm = hp.tile([128, 1], F32, name=f"gm{g}")
                nc.vector.tensor_tensor(gm, ge, qm_g[g], ALU.mult)
                zg = hp.tile([128, 1], F32, name=f"zg{g}")
                nc.vector.tensor_tensor(zg, gm, zf, ALU.mult)
                # per-batch sums via block-ones matmul
                nc.tensor.matmul(psN, bones, zg, start=True, stop=True)
                nc.tensor.matmul(psDen, bones, gm, start=True, stop=True)
                den = hp.tile([4, 1], F32, name=f"den{g}")
                nc.vector.tensor_scalar(den, psDen, EPS, None, ALU.add)
                rec = hp.tile([4, 1], F32, name=f"rec{g}")
                nc.vector.reciprocal(rec, den)
                outv = hp.tile([4, 1], F32, name=f"outv{g}")
                nc.vector.scalar_tensor_tensor(outv, psN, 1.0, rec, ALU.mult, ALU.mult)
                nc.sync.dma_start(out[4 * g : 4 * g + 4].rearrange("(p o) -> p o", o=1), outv)
    return nc


_CACHE = {}


def _get_nc():
    if "nc" not in _CACHE:
        nc = bacc.Bacc("TRN2", target_bir_lowering=False, debug=False)
        build_program(nc)
        nc.compile()
        _CACHE["nc"] = nc
    return _CACHE["nc"]


def kernel(**inputs):
    nc = _get_nc()
    inp = {k: np.ascontiguousarray(np.asarray(v, dtype=np.float32)) for k, v in inputs.items()}
    inp.pop("document_mask", None)
    small = {k: inp[k] for k in ("w1", "b1", "w2", "b2", "w3", "b3", "gw", "gb")}
    in_maps = []
    for i in range(8):
        sl = slice(NB * i, NB * (i + 1))
        m = dict(small)
        m["query"] = inp["query"][sl]
        m["document"] = inp["document"][sl]
        m["query_mask"] = inp["query_mask"][sl]
        m["q_idf"] = inp["q_idf"][sl]
        in_maps.append(m)
    res = run_bass_kernel_spmd(nc, in_maps, core_ids=list(range(8)))
    return np.concatenate([r["out"] for r in res.results]).astype(np.float32)
